# revision 35
# baseline (speedup 1.0000x reference)
"""Nearest-class-mean softmax scores on 8 Trainium2 NeuronCores.

Computes softmax(-(||x||^2 + ||mu||^2 - 2 x.mu)) row-wise for
X:[32768,512], muK:[2048,512], with classes where cK==0 masked to the
per-row min score minus 1 before the softmax.

Key algebraic facts exploited:
  * softmax is invariant to per-row additive shifts, so the ||x||^2 term
    (constant along the class axis) is dropped entirely, as is any global
    constant subtracted from ||mu||^2 (we center m2 to keep fp16 accurate).
  * the masked classes' reference probabilities are exp(min-1-max)/Z which
    underflows to exactly 0.0 in fp32 for this data distribution (row score
    spread is ~300+ while fp32 exp underflows below -87.3). Encoding the
    mask as a -50000 additive score term reproduces exactly those zeros and
    leaves max/Z untouched.
  * probabilities are in [0,1]; fp16 output (upcast on host) halves the
    dominant HBM write traffic at ~5e-4 relative cost.

Device work per core (data-parallel over query rows, muK replicated):
  psum[128,2048]  = (X_tile.T).T @ (2*muK.T)            (PE, fp16 in)
  nsco, nm        = (m2_bc - psum), min-reduce          (DVE ttr: -scores, -max)
  ot16, Z         = exp(-nsco + nm), accum              (ACT: exp(s - max), fp16 out)
  ot16           *= 1/Z                                 (DVE fp16 fast path)
"""

import numpy as np

import concourse.bass as bass
import concourse.tile as tile
from concourse import bacc, mybir
from concourse import dve_ops
from concourse.bass_utils import run_bass_kernel_spmd
from concourse.dve_spec import C0, Spec, Src0, Src1, minn


def _register_rsub_min():
    """Custom DVE op: out = in1 - in0 (elementwise), accum_out = rowmin(out).

    With in0 = psum (2 x.mu) and in1 = m2 broadcast, out is -score and the
    accumulator is -rowmax(score): both the m2 subtraction and the softmax
    max-reduction happen in a single 1x-rate Vector pass over PSUM. Table
    bytes are generated per-NEFF at compile time (no firmware change)."""
    name = "NCM_RSUB_MIN"
    for op in dve_ops.OPS:
        if op.name == name:
            return op

    def _ref(in0, in1, c0, c1, c2):
        b = in1.astype(np.float32) - in0.astype(np.float32)
        mn = b.reshape(b.shape[0], -1).min(axis=-1, keepdims=True)
        return b, np.minimum(np.float32(c0), mn)

    spec = Spec(body=Src1 - Src0, accum=minn, accum_init=C0, reference=_ref)
    op = dve_ops.DveOp(name, spec, subdim=False, uops_sha={})
    dve_ops._SUB_OPCODE_FOR_NAME[name] = (
        max(dve_ops._SUB_OPCODE_FOR_NAME.values()) + 1)
    assert dve_ops._SUB_OPCODE_FOR_NAME[name] < 0x20
    for ver in ("v3",):
        try:
            op.compile(ver)
        except ValueError as e:  # message carries the freshly-computed sha
            import re
            m = re.search(r"\bv\d+: ([0-9a-f]{16})", str(e))
            op.uops_sha[ver] = m.group(1)
            op.compile(ver)
    dve_ops.OPS.append(op)
    dve_ops.CUSTOM_DVE_SPECS[name] = spec
    return op


NCM_RSUB_MIN = _register_rsub_min()

N, C, D = 32768, 2048, 512
NCORES = 8
NS = N // NCORES          # 4096 query rows per core
P = 128                   # partitions
KCH = D // P              # 4 contraction chunks of 128
NB = 512                  # matmul moving free-dim (one PSUM bank)
CCH = C // NB             # 4 output column chunks
MM_DT = mybir.dt.float16  # matmul operand dtype (1 cycle/row on PE)
OUT_DT = mybir.dt.float16 # device output dtype (upcast to f32 on host)
F32 = mybir.dt.float32
MASK_M2 = 50000.0         # m2 value for cK==0 classes -> score -50000 -> exp==0.0f
NTILES = NS // P          # 32 query tiles per core
WARMUP_MMS = 0           # junk N=128 PE-warmup matmuls during the input wait
# xt column-stage widths: small stages first so the head tiles' weights land
# ASAP (stage 0 covers the paired tiles 0-1)
XT_STAGES = [256, 256, 512, 1024, 1024, 1024]
assert sum(XT_STAGES) == NS


def build_nc(ns: int = NS):
    """Build the per-core Bass program (SPMD: same program, per-core inputs)."""
    ntiles = ns // P
    nc = bacc.Bacc("TRN2", target_bir_lowering=False)
    # xt is packed per stage on the host: stage s occupies dram columns
    # [KCH*c0(s), KCH*c1(s)) so each stage DMA moves one contiguous
    # (KCH*width*2)-byte run per partition -- large DMA lines, not the
    # 512 tiny descriptors a strided [P, KCH, ns] slice would produce.
    xt = nc.dram_tensor("xt", [P, KCH * ns], MM_DT, kind="ExternalInput")
    rhs = nc.dram_tensor("rhs", [KCH, P, C], MM_DT, kind="ExternalInput")
    m2r = nc.dram_tensor("m2r", [1, C], F32, kind="ExternalInput")
    out = nc.dram_tensor("out", [ns, C], OUT_DT, kind="ExternalOutput")

    AF = mybir.ActivationFunctionType
    with tile.TileContext(nc) as tc:
        with (
            tc.tile_pool(name="const", bufs=1) as const,
            tc.tile_pool(name="psum", bufs=2, space=bass.MemorySpace.PSUM) as psum,
            tc.tile_pool(name="ss", bufs=4) as ssp,
            tc.tile_pool(name="outp", bufs=4) as outp,
            tc.tile_pool(name="stat", bufs=12) as stat,
        ):
            stage_c0 = []
            c0 = 0
            for w in XT_STAGES:
                stage_c0.append(c0)
                c0 += w
            xts_sb = [const.tile([P, KCH * w], MM_DT, name=f"xts{s}")
                      for s, w in enumerate(XT_STAGES)]
            rhs_sb = [const.tile([P, C], MM_DT, name=f"rhs{k}") for k in range(KCH)]
            m2r_sb = const.tile([1, C], F32, name="m2r_sb")
            m2bc_sb = const.tile([P, C], F32, name="m2bc_sb")

            def stage_dma(s):
                a = KCH * stage_c0[s]
                b = a + KCH * XT_STAGES[s]
                nc.sync.dma_start(xts_sb[s][:], xt[:, a:b])

            def lhsT_of(i, k):
                col = i * P
                s = 0
                while col >= stage_c0[s] + XT_STAGES[s]:
                    s += 1
                w = XT_STAGES[s]
                o = k * w + (col - stage_c0[s])
                return xts_sb[s][:, o:o + P]

            # startup-latency-aware order: the first matmuls need only
            # xt stage 0 + rhs[0][:, 0:512]; everything else streams in
            # behind the compute. m2r is tiny (8 KB) and first; its 16-way
            # completion semaphore confirms slowly (1-partition dest), but
            # the broadcast result is only needed by tile 0's sub pass.
            nc.sync.dma_start(m2r_sb[:], m2r[:])
            stage_dma(0)
            nc.sync.dma_start(rhs_sb[0][:, 0:NB], rhs[0][:, 0:NB])
            nc.sync.dma_start(rhs_sb[0][:, NB:], rhs[0][:, NB:])
            nc.sync.dma_start(rhs_sb[1][:], rhs[1])
            stage_dma(1)
            nc.sync.dma_start(rhs_sb[2][:], rhs[2])
            nc.sync.dma_start(rhs_sb[3][:], rhs[3])
            # on-chip row broadcast: 8 KB from HBM instead of 1 MB
            nc.gpsimd.partition_broadcast(m2bc_sb[:], m2r_sb[:])
            for s in range(2, len(XT_STAGES)):
                stage_dma(s)

            # PE warmup: junk matmuls with no DMA dependency keep the PE's
            # HAM activity monitor busy while the first inputs stream in, so
            # the real tile-0 matmuls start at 2.4 GHz instead of 1.2. Their
            # PSUM writes land in tile 0's psum slot and are discarded by the
            # real k==0 start=True reset.
            wl = const.tile([P, P], MM_DT, name="warm_l")
            wr = const.tile([P, P], MM_DT, name="warm_r")
            nc.gpsimd.memset(wl[:], 0.0)
            nc.gpsimd.memset(wr[:], 0.0)

            h = C // 2

            def post_tile(i, ps):
                """sub/min -> exp -> 1/Z -> scale -> store for one query tile."""
                nsco = ssp.tile([P, C], F32, name="nsco")
                nm = stat.tile([P, 1], F32, name="nm")
                nc.vector._custom_dve(
                    NCM_RSUB_MIN, out=nsco[:], accum_out=nm[:],
                    in0=ps[:, :], in1=m2bc_sb[:], s0=3.0e38,
                )
                # ot = exp(-nsco + nm) = exp(score - max); zs = sum(ot)
                ot = outp.tile([P, C], OUT_DT, name="ot")
                zs = stat.tile([P, 1], F32, name="zs")
                nc.scalar.activation(
                    ot[:], nsco[:], AF.Exp,
                    bias=nm[:], scale=-1.0, accum_out=zs[:],
                )
                rz = stat.tile([P, 1], F32, name="rz")
                nc.vector.reciprocal(rz[:], zs[:])
                nc.vector.tensor_scalar_mul(ot[:], ot[:], rz[:])
                nc.gpsimd.dma_start(out[i * P:(i + 1) * P, :], ot[:])

            # head: tiles 0+1 fused k-outer so each rhs[k] arrival feeds
            # 8 back-to-back matmuls -- bridges the PE through the rhs
            # stream's arrival gaps without HAM re-throttling
            ps_pair = [psum.tile([P, C], F32, name="ps") for _ in range(2)]
            for _ in range(WARMUP_MMS):
                nc.tensor.matmul(
                    ps_pair[0][:, 0:P], wl[:], wr[:], start=True, stop=True)
            for k in range(KCH):
                for t in range(2):
                    lhsT = lhsT_of(t, k)
                    for c in range(CCH):
                        nc.tensor.matmul(
                            ps_pair[t][:, c * NB:(c + 1) * NB],
                            lhsT,
                            rhs_sb[k][:, c * NB:(c + 1) * NB],
                            start=(k == 0),
                            stop=(k == KCH - 1),
                        )
            post_tile(0, ps_pair[0])
            post_tile(1, ps_pair[1])

            for i in range(2, ntiles):
                last = i == ntiles - 1
                if not last:
                    ps = psum.tile([P, C], F32, name="ps")
                    for k in range(KCH):
                        lhsT = lhsT_of(i, k)
                        for c in range(CCH):
                            nc.tensor.matmul(
                                ps[:, c * NB:(c + 1) * NB],
                                lhsT,
                                rhs_sb[k][:, c * NB:(c + 1) * NB],
                                start=(k == 0),
                                stop=(k == KCH - 1),
                            )
                    post_tile(i, ps)
                else:
                    # drain-latency-aware last tile: c-outer matmul halves in
                    # two separate PSUM tiles (PSUM deps are tile-granular) so
                    # the first sub/min pass hides under the second half's
                    # matmuls; final stores go out on the two HWDGE rings
                    # (sync + scalar) for fast completion receipt.
                    lhsTs = [lhsT_of(i, k) for k in range(KCH)]
                    nsco = ssp.tile([P, C], F32)
                    nmh = [stat.tile([P, 1], F32, name=f"nmh{j}") for j in range(2)]
                    zsh = [stat.tile([P, 1], F32, name=f"zsh{j}") for j in range(2)]
                    psh = [psum.tile([P, C], F32, name="ps") for j in range(2)]
                    for half in range(2):
                        cs = (0, 1) if half == 0 else (2, 3)
                        for k in range(KCH):
                            for ci, c in enumerate(cs):
                                nc.tensor.matmul(
                                    psh[half][:, ci * NB:(ci + 1) * NB],
                                    lhsTs[k],
                                    rhs_sb[k][:, c * NB:(c + 1) * NB],
                                    start=(k == 0),
                                    stop=(k == KCH - 1),
                                )
                        sl = slice(half * h, (half + 1) * h)
                        nc.vector._custom_dve(
                            NCM_RSUB_MIN, out=nsco[:, sl], accum_out=nmh[half][:],
                            in0=psh[half][:, 0:h], in1=m2bc_sb[:, sl], s0=3.0e38,
                        )
                    nm = stat.tile([P, 1], F32)
                    nc.vector.tensor_tensor(
                        nm[:], nmh[0][:], nmh[1][:], op=mybir.AluOpType.min)
                    ot = outp.tile([P, C], OUT_DT)
                    zs = stat.tile([P, 1], F32)
                    nc.scalar.activation(
                        ot[:], nsco[:], AF.Exp,
                        bias=nm[:], scale=-1.0, accum_out=zs[:],
                    )
                    rz = stat.tile([P, 1], F32)
                    nc.vector.reciprocal(rz[:], zs[:])
                    # scale+store in quarters, alternating the two HWDGE
                    # rings, so the final store issues (and its completion
                    # receipt starts) as early as possible
                    q = C // 4
                    for j in range(4):
                        sl = slice(j * q, (j + 1) * q)
                        nc.vector.tensor_scalar_mul(ot[:, sl], ot[:, sl], rz[:])
                        eng = nc.sync if j % 2 == 0 else nc.scalar
                        eng.dma_start(out[i * P:(i + 1) * P, sl], ot[:, sl])

    nc.compile()
    return nc


_NC_CACHE = {}


def _get_nc(ns: int = NS):
    if ns not in _NC_CACHE:
        _NC_CACHE[ns] = build_nc(ns)
    return _NC_CACHE[ns]


def prep_inputs(X, muK, cK):
    """Host-side shard/layout prep (numpy only)."""
    X = np.asarray(X, dtype=np.float32)
    muK = np.asarray(muK, dtype=np.float32)
    cK = np.asarray(cK, dtype=np.float32)

    m2 = np.sum(muK.astype(np.float64) ** 2, axis=1)
    m2c = m2 - m2.mean()  # centered: softmax-invariant shift
    m2m = np.where(cK == 0.0, MASK_M2, m2c).astype(np.float32)
    m2r_np = np.ascontiguousarray(m2m.reshape(1, C))
    rhs_np = np.ascontiguousarray(
        (2.0 * muK.T).astype(np.float16).reshape(KCH, P, C))
    Xt = X.T.astype(np.float16)  # [D, N]

    in_maps = []
    for core in range(NCORES):
        xs = Xt[:, core * NS:(core + 1) * NS]              # [D, NS]
        xs = xs.reshape(KCH, P, NS).transpose(1, 0, 2)     # [P, KCH, NS]
        # pack per stage: [P, sum_s KCH*w_s] with each stage contiguous
        parts = []
        c0 = 0
        for w in XT_STAGES:
            parts.append(xs[:, :, c0:c0 + w].reshape(P, KCH * w))
            c0 += w
        xp = np.ascontiguousarray(np.concatenate(parts, axis=1))
        in_maps.append({"xt": xp, "rhs": rhs_np, "m2r": m2r_np})
    return in_maps


def run(X, muK, cK, trace=False, **kw):
    in_maps = prep_inputs(X, muK, cK)
    nc = _get_nc()
    res = run_bass_kernel_spmd(
        nc, in_maps, list(range(NCORES)), trace=trace, **kw)
    full = np.concatenate(
        [res.results[c]["out"] for c in range(NCORES)], axis=0)
    return full.astype(np.float32), res


def kernel(X, muK, cK):
    full, _ = run(X, muK, cK, trace=False)
    return full
